# revision 13
# baseline (speedup 1.0000x reference)
"""Trainium2 Bass kernel for nn_Autoregression — fp8 DoubleRow version.

Math: log_prob[b,k,t] = -0.5*(C*log(2pi) + logdet(Sigma_k)
+ ||L_k^{-1}(conv(x,W_k)+b_k)||^2).  Fold L^{-1} into the conv (W2, b2),
then mahal = sum_c es^2 with es = conv(x; W2, b2).

Device layout (per core, T sharded 8 ways, 8-sample left halo):
es computed as [kc, t] PSUM tiles (kc = 2 states x 64 ch per block, 8
blocks) so the channel reduction runs on the PE.  Conv contraction 577 =
(64ci x 9taps + bias) done in 3 fp8 DoubleRow matmuls per block per
512-t block (2x128-row k-tiles each, weights stationary, x moving with
a stride-2 overlapped AP).  Act squares groups 0-2 (PSUM -> fp8 SBUF);
group 3 is copied to bf16 SBUF by DVE and squared to fp8 by GpSimd
(DVE TensorTensor cannot read PSUM on both ports; walrus rejects it).
The tap-8+bias step uses a full 128-partition xed tile (row 64 = ones,
rows 65-127 = zeros w/ zero weights): 65-partition DoubleRow wedges the
device.  Its second k-tile points at a fixed zero dead-zone column
range (zero weights), keeping the AP a plain strided one.
4 DoubleRow mask matmuls reduce 64-channel groups -> mahal PSUM
[16, t]; DVE tensor_scalar applies -0.5*x+bias -> out SBUF f32;
sync-ring DMA out.  PE waits are kept single-semaphore via observer
matmuls (baseline trick).
"""

import math
import os

import numpy as np
import ml_dtypes

import concourse.bass as bass
import concourse.bacc as bacc_mod
import concourse.mybir as mybir
import concourse.tile as tile
from concourse.bass_utils import run_bass_kernel_spmd
from concourse.tile_rust import add_dep_helper
import bass_rust

K = 16
C = 64
T = 65536
AR = 8
NCORES = 8
TLOC = T // NCORES
TB = 512                    # t per block-iteration
NB = 8                      # kc blocks (2 states x 64ch each)
NG = 4                      # es groups per t-block (2 kc blocks each)
NS = 3                      # conv contraction steps (256+256+65)

FP8 = mybir.dt.float8e4
_FP8_NP = ml_dtypes.float8_e4m3

_CACHE: dict = {}


def _chunks(tloc):
    """xin/xe DMA chunks: [0, 520), then 4-t-block strides with halo."""
    ntb = tloc // TB
    bnds = [0, 1, 2] + [2 + 4 * i for i in range(1, (ntb + 2) // 4)] + [ntb]
    bnds = sorted(set(b for b in bnds if b <= ntb))
    out = []
    for a, b in zip(bnds[:-1], bnds[1:]):
        lo = a * TB
        hi = (b - 1) * TB + TB + AR  # last col read: (b-1)*TB + 4s+2i+511 <= +519
        out.append((a, lo, min(hi, tloc + AR + 1)))
    return out


def _build_program(tloc=TLOC):
    nc = bacc_mod.Bacc()
    f32 = mybir.dt.float32
    ntb = tloc // TB

    xin = nc.declare_dram_parameter("xin", [128, tloc + AR + 1], FP8, isOutput=False)
    wts = nc.declare_dram_parameter("wts", [128, 6 * NB, 128], FP8, isOutput=False)
    maskd = nc.declare_dram_parameter("maskd", [128, 8, K], FP8, isOutput=False)
    biasd = nc.declare_dram_parameter("biasd", [K, 1], f32, isOutput=False)
    xed = nc.declare_dram_parameter("xed", [128, tloc + TB], FP8, isOutput=False)
    out = nc.declare_dram_parameter("out", [K, tloc], f32, isOutput=True)

    chunks = _chunks(tloc)

    with tile.TileContext(nc) as tc:
        with (
            tc.tile_pool(name="singles", bufs=1) as singles,
            tc.tile_pool(name="sqpool", bufs=2) as sqpool,
            tc.tile_pool(name="esbpool", bufs=2) as esbpool,
            tc.tile_pool(name="es_ps", bufs=3, space="PSUM") as es_ps,
            tc.tile_pool(name="m_ps", bufs=1, space="PSUM") as m_ps,
            tc.tile_pool(name="obs_ps", bufs=1, space="PSUM") as obs_ps,
        ):
            # --- observer machinery (single-sem matmul waits; see baseline) ---
            scratch = obs_ps.tile([2, 128], f32)
            pending = []

            def pe_observe(col):
                i = nc.tensor.matmul(
                    scratch[0:2, 0:2], col, col, start=True, stop=True
                )
                pending.append(i)

            def _flush(i):
                while pending:
                    add_dep_helper(i.ins, pending.pop().ins, sync=False)
                return i

            # --- SBUF tiles ---
            w_sb = singles.tile([128, 6 * NB, 128], FP8)
            mask_sb = singles.tile([128, 8, K], FP8)
            bias_sb = singles.tile([K, 1], f32)
            dummy_sb = singles.tile([K, 1], f32)
            xin_sb = singles.tile([128, tloc + AR + 1], FP8)
            xed_sb = singles.tile([128, tloc + TB], FP8)
            out_sb = singles.tile([K, tloc], f32)

            # --- input DMAs: all on the sync ring (keeps Act/DVE/Pool
            # queues free of DMA issue cost).  Critical-path order first.
            a0, lo0, hi0 = chunks[0]
            nc.sync.dma_start(out=w_sb[:, 0:1, :], in_=wts[:, 0:1, :])
            # warmups: ramp PE/HAM while the rest of the DMAs land (these
            # wait only on the tiny 16KB w slice just above)
            for _ in range(22):
                nc.tensor.matmul(
                    scratch[0:2, 0:128],
                    w_sb[:, 0, 0:2],
                    w_sb[:, 0, 0:128],
                    start=True,
                    stop=True,
                )
            nc.sync.dma_start(out=bias_sb, in_=biasd[:, :])
            nc.scalar.dma_start(out=xin_sb[:, lo0:hi0], in_=xin[:, lo0:hi0])
            nc.gpsimd.dma_start(
                out=xed_sb[:, lo0 : min(hi0, tloc)], in_=xed[:, lo0 : min(hi0, tloc)]
            )
            nc.gpsimd.dma_start(out=xed_sb[:, tloc:], in_=xed[:, tloc:])
            # weights in per-t-block consumption order: g3 (b6-7) first
            nc.sync.dma_start(out=w_sb[:, 36:48, :], in_=wts[:, 36:48, :])
            nc.sync.dma_start(out=w_sb[:, 1:6, :], in_=wts[:, 1:6, :])
            nc.sync.dma_start(out=w_sb[:, 6:12, :], in_=wts[:, 6:12, :])
            nc.scalar.dma_start(out=mask_sb, in_=maskd[:, :, :])
            for bb in (2, 4):
                nc.sync.dma_start(
                    out=w_sb[:, 6 * bb : 6 * bb + 12, :],
                    in_=wts[:, 6 * bb : 6 * bb + 12, :],
                )
            for a, lo, hi in chunks[1:]:
                nc.scalar.dma_start(out=xin_sb[:, lo:hi], in_=xin[:, lo:hi])
                he = min(hi, tloc)
                nc.gpsimd.dma_start(out=xed_sb[:, lo:he], in_=xed[:, lo:he])

            # DVE: touch bias first (absorbs its DMA sem on DVE's in-order
            # stream before the first affine tensor_scalar needs it)
            nc.vector.tensor_copy(dummy_sb, bias_sb)

            chunk_starts = {a: i for i, (a, lo, hi) in enumerate(chunks)}

            def conv_rhs(tb, s):
                """moving x AP for step s: [128(p), 2(ktile), TB] overlapped."""
                off = tb * TB
                if s < 2:
                    ap = xin_sb[:, off + 4 * s : off + 4 * s + TB].copy()
                    p = ap.ap[0]
                    ap.ap = bass_rust.VecI64Pair([[p[0], p[1]], [2, 2], [1, TB]])
                else:
                    # tile0 = tap8+bias cols; tile1 = fixed zero dead zone at
                    # [tloc, tloc+TB) (zero weights) via static stride
                    ap = xed_sb[:, off : off + TB].copy()
                    p = ap.ap[0]
                    ap.ap = bass_rust.VecI64Pair(
                        [[p[0], p[1]], [tloc - off, 2], [1, TB]]
                    )
                return ap

            def conv_lhsT(b, s):
                if s < 2:
                    return w_sb[:, 6 * b + 2 * s : 6 * b + 2 * s + 2, :]
                return w_sb[:, 6 * b + 4 : 6 * b + 6, :]

            DR = mybir.MatmulPerfMode.DoubleRow
            mlast = {}

            def emit_masks(tb):
                m = m_ps.tile([K, TB], f32, name="m", tag="m")
                sq = sqtiles[tb]
                for p in range(4):
                    i = nc.tensor.matmul(
                        m[:, :],
                        mask_sb[:, 2 * p : 2 * p + 2, :],
                        sq[:, 2 * p : 2 * p + 2, :],
                        start=(p == 0),
                        stop=(p == 3),
                        perf_mode=DR,
                    )
                    if p == 0:
                        _flush(i)
                mlast[tb] = m

            def emit_affine(tb):
                nc.vector.tensor_scalar(
                    out=out_sb[:, tb * TB : (tb + 1) * TB],
                    in0=mlast.pop(tb)[:, :],
                    scalar1=-0.5,
                    scalar2=bias_sb,
                    op0=mybir.AluOpType.mult,
                    op1=mybir.AluOpType.add,
                )

            sqtiles = {}
            for tb in range(ntb):
                sq = sqpool.tile([128, NB, TB], FP8, name="sq", tag="sq")
                sqtiles[tb] = sq
                gorder = (3, 0, 1, 2)
                for gi, g in enumerate(gorder):
                    es = es_ps.tile([128, 2, TB], f32, name="es", tag="es")
                    if gi == 0:
                        if tb in chunk_starts:
                            off = tb * TB
                            pe_observe(xin_sb[:, off : off + 2])
                            pe_observe(xed_sb[:, off : off + 2])
                        if tb == 0:
                            pe_observe(w_sb[:, 0, 0:2])
                            pe_observe(xed_sb[:, tloc : tloc + 2])
                    for h in range(2):
                        b = 2 * g + h
                        if tb == 0 and b >= 1:
                            pe_observe(w_sb[:, 6 * b, 0:2])
                        for s in range(NS):
                            i = nc.tensor.matmul(
                                es[:, h, :],
                                conv_lhsT(b, s),
                                conv_rhs(tb, s),
                                start=(s == 0),
                                stop=(s == 2),
                                perf_mode=DR,
                            )
                            if s == 0:
                                _flush(i)
                    with nc.allow_low_precision(
                        reason="squares quantized to fp8; validated host-side "
                        "(rel err 1.5e-2 vs 2e-2 budget)"
                    ):
                        if g < 3:
                            nc.scalar.activation(
                                sq[:, 2 * g : 2 * g + 2, :],
                                es[:, :, :],
                                mybir.ActivationFunctionType.Square,
                            )
                        else:
                            esb = esbpool.tile(
                                [128, 2, TB], mybir.dt.bfloat16, name="esb", tag="esb"
                            )
                            nc.vector.tensor_copy(esb, es[:, :, :])
                            nc.gpsimd.tensor_tensor(
                                sq[:, 6:8, :], esb, esb, mybir.AluOpType.mult
                            )
                    if gi == 1 and tb > 0:
                        if tb == 1:
                            pe_observe(mask_sb[:, 0, 0:2])
                        else:
                            # absorb the DVE affine(tb-2) sem (m-tile WAR)
                            t2 = (tb - 2) * TB
                            pe_observe(out_sb[0:2, t2 : t2 + 2])
                        emit_masks(tb - 1)
                    if gi == 2 and tb > 0:
                        emit_affine(tb - 1)
                    if gi == 3 and tb > 0:
                        t0 = (tb - 1) * TB
                        nc.sync.dma_start(
                            out=out[:, t0 : t0 + TB], in_=out_sb[:, t0 : t0 + TB]
                        )
            t2 = (ntb - 2) * TB
            pe_observe(out_sb[0:2, t2 : t2 + 2])
            emit_masks(ntb - 1)
            emit_affine(ntb - 1)
            t0 = (ntb - 1) * TB
            nc.sync.dma_start(out=out[:, t0 : t0 + TB], in_=out_sb[:, t0 : t0 + TB])
    nc.compile()
    return nc


def _prep_host(W, b, Sigma):
    """Fold L^{-1} into conv weights; pack fp8 DoubleRow tiles + constants."""
    W64 = W.astype(np.float64)
    b64 = b.astype(np.float64)
    S64 = Sigma.astype(np.float64)
    L = np.linalg.cholesky(S64)
    Li = np.linalg.inv(L)
    logdet = 2.0 * np.sum(np.log(np.diagonal(L, axis1=1, axis2=2)), axis=1)
    W2 = np.einsum("kdc,kcij->kdij", Li, W64)   # [K, d, ci, 9]
    b2 = np.einsum("kdc,kc->kd", Li, b64)       # [K, d]

    W2q = W2.astype(np.float32).astype(_FP8_NP).astype(np.float32)
    b2q = b2.astype(np.float32).astype(_FP8_NP).astype(np.float32)

    # w_np[r, 6b+2s+i, m]: m = 64*(k-2b) + d
    #   s<2: = W2[2b + m//64, m%64, r%64, 4s+2i + r//64]
    #   s=2,i=0: r<64 -> W2[.., r, 8]; r==64 -> b2; else 0.  i=1: 0
    w_np = np.zeros((128, 48, 128), np.float32)
    # [k, d, ci, j] -> view [b, kin2, d, ci, j]
    Wb = W2q.reshape(NB, 2, C, C, 9)
    for b_ in range(NB):
        for s in range(2):
            for i in range(2):
                for par in range(2):
                    j = 4 * s + 2 * i + par
                    # rows par*64 + ci, cols kin2*64 + d
                    blk = Wb[b_, :, :, :, j]              # [kin2, d, ci]
                    w_np[par * C : par * C + C, 6 * b_ + 2 * s + i, :] = (
                        blk.transpose(2, 0, 1).reshape(C, 128)
                    )
        w_np[0:C, 6 * b_ + 4, :] = (
            Wb[b_, :, :, :, 8].transpose(2, 0, 1).reshape(C, 128)
        )
        w_np[C, 6 * b_ + 4, :] = b2q.reshape(NB, 128)[b_]

    mask_np = np.zeros((128, 8, K), np.float32)
    r = np.arange(128)
    for p in range(4):
        for i in range(2):
            mask_np[r, 2 * p + i, 4 * p + 2 * i + r // C] = 1.0

    const = C * math.log(2.0 * math.pi) + logdet
    bias_np = (-0.5 * const).astype(np.float32).reshape(K, 1)
    return w_np.astype(_FP8_NP), mask_np.astype(_FP8_NP), bias_np


def _make_in_maps(x, w_np, mask_np, bias_np, tloc=TLOC, ncores=NCORES):
    xq = np.asarray(x, np.float32)[0].astype(_FP8_NP).astype(np.float32)
    xpad = np.pad(xq, ((0, 0), (AR, TB + 2)))       # [C, AR+T+TB+2]
    in_maps = []
    for i in range(ncores):
        lo = xpad[:, tloc * i : tloc * i + tloc + AR + 1]
        hi = xpad[:, tloc * i + 1 : tloc * i + tloc + AR + 2]
        # xed col u = x_glob[core_start + u]; row 64 ones; rows 65-127 zeros;
        # cols [tloc, tloc+TB) zero dead-zone for the s2 second k-tile
        xed_np = np.zeros((128, tloc + TB), np.float32)
        xed_np[0:C, 0:tloc] = xpad[:, tloc * i + AR : tloc * i + AR + tloc]
        xed_np[C, :] = 1.0
        xed_np[:, tloc:] = 0.0
        in_maps.append(
            {
                "xin": np.ascontiguousarray(
                    np.concatenate([lo, hi], axis=0).astype(_FP8_NP)
                ),
                "xed": xed_np.astype(_FP8_NP),
                "wts": w_np,
                "maskd": mask_np,
                "biasd": bias_np,
            }
        )
    return in_maps


def _run(x, W, b, Sigma, trace=False):
    if "nc" not in _CACHE:
        _CACHE["nc"] = _build_program()
    nc = _CACHE["nc"]
    w_np, mask_np, bias_np = _prep_host(
        np.asarray(W, np.float32), np.asarray(b, np.float32),
        np.asarray(Sigma, np.float32),
    )
    in_maps = _make_in_maps(np.asarray(x, np.float32), w_np, mask_np, bias_np)
    res = run_bass_kernel_spmd(
        nc, in_maps, core_ids=list(range(NCORES)), trace=trace
    )
    outs = [res.results[i]["out"] for i in range(NCORES)]
    full = np.concatenate(outs, axis=1)[None]   # [1, K, T]
    return full.astype(np.float32), res


def kernel(x, W, b, Sigma):
    out, _ = _run(x, W, b, Sigma, trace=bool(int(os.environ.get("BASS_TRACE", "0"))))
    return out


# revision 14
# speedup vs baseline: 1.1961x; 1.1961x over previous
"""Trainium2 Bass kernel for nn_Autoregression — fp8 DoubleRow version.

Math: log_prob[b,k,t] = -0.5*(C*log(2pi) + logdet(Sigma_k)
+ ||L_k^{-1}(conv(x,W_k)+b_k)||^2).  Fold L^{-1} into the conv (W2, b2),
then mahal = sum_c es^2 with es = conv(x; W2, b2).

Device layout (per core, T sharded 8 ways, 8-sample left halo):
es computed as [kc, t] PSUM tiles (kc = 2 states x 64 ch per block, 8
blocks) so the channel reduction runs on the PE.  Conv contraction 577 =
(64ci x 9taps + bias) done in 3 fp8 DoubleRow matmuls per block per
512-t block (2x128-row k-tiles each, weights stationary, x moving with
a stride-2 overlapped AP).  Act squares groups 0-2 (PSUM -> fp8 SBUF);
group 3 is copied to bf16 SBUF by DVE and squared to fp8 by GpSimd
(DVE TensorTensor cannot read PSUM on both ports; walrus rejects it).
The tap-8+bias step uses a full 128-partition xed tile (row 64 = ones,
rows 65-127 = zeros w/ zero weights): 65-partition DoubleRow wedges the
device.  Its second k-tile points at a fixed zero dead-zone column
range (zero weights), keeping the AP a plain strided one.
4 DoubleRow mask matmuls reduce 64-channel groups -> mahal PSUM
[16, t]; DVE tensor_scalar applies -0.5*x+bias -> out SBUF f32;
sync-ring DMA out.  PE waits are kept single-semaphore via observer
matmuls (baseline trick).
"""

import math
import os

import numpy as np
import ml_dtypes

import concourse.bass as bass
import concourse.bacc as bacc_mod
import concourse.mybir as mybir
import concourse.tile as tile
from concourse.bass_utils import run_bass_kernel_spmd
from concourse.tile_rust import add_dep_helper
import bass_rust

K = 16
C = 64
T = 65536
AR = 8
NCORES = 8
TLOC = T // NCORES
TB = 512                    # t per block-iteration
NB = 8                      # kc blocks (2 states x 64ch each)
NG = 4                      # es groups per t-block (2 kc blocks each)
NS = 3                      # conv contraction steps (256+256+65)

FP8 = mybir.dt.float8e4
_FP8_NP = ml_dtypes.float8_e4m3

_CACHE: dict = {}


def _chunks(tloc):
    """xin/xe DMA chunks: [0, 520), then 4-t-block strides with halo."""
    ntb = tloc // TB
    bnds = [0, 1, 2] + [2 + 4 * i for i in range(1, (ntb + 2) // 4)] + [ntb]
    bnds = sorted(set(b for b in bnds if b <= ntb))
    out = []
    for a, b in zip(bnds[:-1], bnds[1:]):
        lo = a * TB
        hi = (b - 1) * TB + TB + AR  # last col read: (b-1)*TB + 4s+2i+511 <= +519
        out.append((a, lo, min(hi, tloc + AR + 1)))
    return out


def _build_program(tloc=TLOC):
    nc = bacc_mod.Bacc()
    f32 = mybir.dt.float32
    ntb = tloc // TB

    xin = nc.declare_dram_parameter("xin", [128, tloc + AR + 1], FP8, isOutput=False)
    wts = nc.declare_dram_parameter("wts", [128, 6 * NB, 128], FP8, isOutput=False)
    maskd = nc.declare_dram_parameter("maskd", [128, 8, K], FP8, isOutput=False)
    biasd = nc.declare_dram_parameter("biasd", [K, 1], f32, isOutput=False)
    xed = nc.declare_dram_parameter("xed", [128, tloc + TB], FP8, isOutput=False)
    out = nc.declare_dram_parameter("out", [K, tloc], f32, isOutput=True)

    chunks = _chunks(tloc)

    with tile.TileContext(nc) as tc:
        with (
            tc.tile_pool(name="singles", bufs=1) as singles,
            tc.tile_pool(name="sqpool", bufs=2) as sqpool,
            tc.tile_pool(name="esbpool", bufs=2) as esbpool,
            tc.tile_pool(name="es_ps", bufs=3, space="PSUM") as es_ps,
            tc.tile_pool(name="m_ps", bufs=1, space="PSUM") as m_ps,
            tc.tile_pool(name="obs_ps", bufs=1, space="PSUM") as obs_ps,
        ):
            # --- observer machinery (single-sem matmul waits; see baseline) ---
            scratch = obs_ps.tile([2, 128], f32)
            pending = []

            def pe_observe(col):
                i = nc.tensor.matmul(
                    scratch[0:2, 0:2], col, col, start=True, stop=True
                )
                pending.append(i)

            def _flush(i):
                while pending:
                    add_dep_helper(i.ins, pending.pop().ins, sync=False)
                return i

            # --- SBUF tiles ---
            w_sb = singles.tile([128, 6 * NB, 128], FP8)
            mask_sb = singles.tile([128, 8, K], FP8)
            bias_sb = singles.tile([K, 1], f32)
            dummy_sb = singles.tile([K, 1], f32)
            xin_sb = singles.tile([128, tloc + AR + 1], FP8)
            xed_sb = singles.tile([128, tloc + TB], FP8)
            out_sb = singles.tile([K, tloc], f32)

            # --- input DMAs: all on the sync ring (keeps Act/DVE/Pool
            # queues free of DMA issue cost).  Critical-path order first.
            a0, lo0, hi0 = chunks[0]
            nc.sync.dma_start(out=bias_sb, in_=biasd[:, :])
            nc.sync.dma_start(out=w_sb[:, 0:1, :], in_=wts[:, 0:1, :])
            # warmups: ramp PE/HAM while the rest of the DMAs land (these
            # wait only on the tiny 16KB w slice just above)
            for _ in range(14):
                nc.tensor.matmul(
                    scratch[0:2, 0:128],
                    w_sb[:, 0, 0:2],
                    w_sb[:, 0, 0:128],
                    start=True,
                    stop=True,
                )
            nc.scalar.dma_start(out=xin_sb[:, lo0:hi0], in_=xin[:, lo0:hi0])
            nc.gpsimd.dma_start(
                out=xed_sb[:, lo0 : min(hi0, tloc)], in_=xed[:, lo0 : min(hi0, tloc)]
            )
            nc.gpsimd.dma_start(out=xed_sb[:, tloc:], in_=xed[:, tloc:])
            # weights in per-t-block consumption order: g3 (b6-7) first
            nc.sync.dma_start(out=w_sb[:, 36:48, :], in_=wts[:, 36:48, :])
            nc.sync.dma_start(out=w_sb[:, 1:6, :], in_=wts[:, 1:6, :])
            nc.sync.dma_start(out=w_sb[:, 6:12, :], in_=wts[:, 6:12, :])
            nc.scalar.dma_start(out=mask_sb, in_=maskd[:, :, :])
            for bb in (2, 4):
                nc.sync.dma_start(
                    out=w_sb[:, 6 * bb : 6 * bb + 12, :],
                    in_=wts[:, 6 * bb : 6 * bb + 12, :],
                )
            for a, lo, hi in chunks[1:]:
                nc.scalar.dma_start(out=xin_sb[:, lo:hi], in_=xin[:, lo:hi])
                he = min(hi, tloc)
                nc.gpsimd.dma_start(out=xed_sb[:, lo:he], in_=xed[:, lo:he])

            # DVE: touch bias first (absorbs its DMA sem on DVE's in-order
            # stream before the first affine tensor_scalar needs it)
            nc.vector.tensor_copy(dummy_sb, bias_sb)

            chunk_starts = {a: i for i, (a, lo, hi) in enumerate(chunks)}

            def conv_rhs(tb, s):
                """moving x AP for step s: [128(p), 2(ktile), TB] overlapped."""
                off = tb * TB
                if s < 2:
                    ap = xin_sb[:, off + 4 * s : off + 4 * s + TB].copy()
                    p = ap.ap[0]
                    ap.ap = bass_rust.VecI64Pair([[p[0], p[1]], [2, 2], [1, TB]])
                else:
                    # tile0 = tap8+bias cols; tile1 = fixed zero dead zone at
                    # [tloc, tloc+TB) (zero weights) via static stride
                    ap = xed_sb[:, off : off + TB].copy()
                    p = ap.ap[0]
                    ap.ap = bass_rust.VecI64Pair(
                        [[p[0], p[1]], [tloc - off, 2], [1, TB]]
                    )
                return ap

            def conv_lhsT(b, s):
                if s < 2:
                    return w_sb[:, 6 * b + 2 * s : 6 * b + 2 * s + 2, :]
                return w_sb[:, 6 * b + 4 : 6 * b + 6, :]

            DR = mybir.MatmulPerfMode.DoubleRow
            mlast = {}

            def emit_masks(tb):
                m = m_ps.tile([K, TB], f32, name="m", tag="m")
                sq = sqtiles[tb]
                for p in range(4):
                    i = nc.tensor.matmul(
                        m[:, :],
                        mask_sb[:, 2 * p : 2 * p + 2, :],
                        sq[:, 2 * p : 2 * p + 2, :],
                        start=(p == 0),
                        stop=(p == 3),
                        perf_mode=DR,
                    )
                    if p == 0:
                        _flush(i)
                mlast[tb] = m

            def emit_affine(tb):
                nc.vector.tensor_scalar(
                    out=out_sb[:, tb * TB : (tb + 1) * TB],
                    in0=mlast.pop(tb)[:, :],
                    scalar1=-0.5,
                    scalar2=bias_sb,
                    op0=mybir.AluOpType.mult,
                    op1=mybir.AluOpType.add,
                )

            sqtiles = {}
            for tb in range(ntb):
                sq = sqpool.tile([128, NB, TB], FP8, name="sq", tag="sq")
                sqtiles[tb] = sq
                gorder = (3, 0, 1, 2)
                for gi, g in enumerate(gorder):
                    es = es_ps.tile([128, 2, TB], f32, name="es", tag="es")
                    if gi == 0:
                        if tb in chunk_starts:
                            off = tb * TB
                            pe_observe(xin_sb[:, off : off + 2])
                            pe_observe(xed_sb[:, off : off + 2])
                        if tb == 0:
                            pe_observe(w_sb[:, 0, 0:2])
                            pe_observe(xed_sb[:, tloc : tloc + 2])
                    for h in range(2):
                        b = 2 * g + h
                        if tb == 0 and b >= 1:
                            pe_observe(w_sb[:, 6 * b, 0:2])
                        for s in range(NS):
                            i = nc.tensor.matmul(
                                es[:, h, :],
                                conv_lhsT(b, s),
                                conv_rhs(tb, s),
                                start=(s == 0),
                                stop=(s == 2),
                                perf_mode=DR,
                            )
                            if s == 0:
                                _flush(i)
                    with nc.allow_low_precision(
                        reason="squares quantized to fp8; validated host-side "
                        "(rel err 1.5e-2 vs 2e-2 budget)"
                    ):
                        if g < 3:
                            nc.scalar.activation(
                                sq[:, 2 * g : 2 * g + 2, :],
                                es[:, :, :],
                                mybir.ActivationFunctionType.Square,
                            )
                        else:
                            esb = esbpool.tile(
                                [128, 2, TB], mybir.dt.bfloat16, name="esb", tag="esb"
                            )
                            nc.vector.tensor_copy(esb, es[:, :, :])
                            nc.gpsimd.tensor_tensor(
                                sq[:, 6:8, :], esb, esb, mybir.AluOpType.mult
                            )
                    if gi == 1 and tb > 0:
                        if tb == 1:
                            pe_observe(mask_sb[:, 0, 0:2])
                        else:
                            # absorb the DVE affine(tb-2) sem (m-tile WAR)
                            t2 = (tb - 2) * TB
                            pe_observe(out_sb[0:2, t2 : t2 + 2])
                        emit_masks(tb - 1)
                    if gi == 2 and tb > 0:
                        emit_affine(tb - 1)
                    if gi == 3 and tb > 0:
                        t0 = (tb - 1) * TB
                        nc.sync.dma_start(
                            out=out[:, t0 : t0 + TB], in_=out_sb[:, t0 : t0 + TB]
                        )
            t2 = (ntb - 2) * TB
            pe_observe(out_sb[0:2, t2 : t2 + 2])
            emit_masks(ntb - 1)
            emit_affine(ntb - 1)
            t0 = (ntb - 1) * TB
            nc.sync.dma_start(out=out[:, t0 : t0 + TB], in_=out_sb[:, t0 : t0 + TB])
    nc.compile()
    return nc


def _prep_host(W, b, Sigma):
    """Fold L^{-1} into conv weights; pack fp8 DoubleRow tiles + constants."""
    W64 = W.astype(np.float64)
    b64 = b.astype(np.float64)
    S64 = Sigma.astype(np.float64)
    L = np.linalg.cholesky(S64)
    Li = np.linalg.inv(L)
    logdet = 2.0 * np.sum(np.log(np.diagonal(L, axis1=1, axis2=2)), axis=1)
    W2 = np.einsum("kdc,kcij->kdij", Li, W64)   # [K, d, ci, 9]
    b2 = np.einsum("kdc,kc->kd", Li, b64)       # [K, d]

    W2q = W2.astype(np.float32).astype(_FP8_NP).astype(np.float32)
    b2q = b2.astype(np.float32).astype(_FP8_NP).astype(np.float32)

    # w_np[r, 6b+2s+i, m]: m = 64*(k-2b) + d
    #   s<2: = W2[2b + m//64, m%64, r%64, 4s+2i + r//64]
    #   s=2,i=0: r<64 -> W2[.., r, 8]; r==64 -> b2; else 0.  i=1: 0
    w_np = np.zeros((128, 48, 128), np.float32)
    # [k, d, ci, j] -> view [b, kin2, d, ci, j]
    Wb = W2q.reshape(NB, 2, C, C, 9)
    for b_ in range(NB):
        for s in range(2):
            for i in range(2):
                for par in range(2):
                    j = 4 * s + 2 * i + par
                    # rows par*64 + ci, cols kin2*64 + d
                    blk = Wb[b_, :, :, :, j]              # [kin2, d, ci]
                    w_np[par * C : par * C + C, 6 * b_ + 2 * s + i, :] = (
                        blk.transpose(2, 0, 1).reshape(C, 128)
                    )
        w_np[0:C, 6 * b_ + 4, :] = (
            Wb[b_, :, :, :, 8].transpose(2, 0, 1).reshape(C, 128)
        )
        w_np[C, 6 * b_ + 4, :] = b2q.reshape(NB, 128)[b_]

    mask_np = np.zeros((128, 8, K), np.float32)
    r = np.arange(128)
    for p in range(4):
        for i in range(2):
            mask_np[r, 2 * p + i, 4 * p + 2 * i + r // C] = 1.0

    const = C * math.log(2.0 * math.pi) + logdet
    bias_np = (-0.5 * const).astype(np.float32).reshape(K, 1)
    return w_np.astype(_FP8_NP), mask_np.astype(_FP8_NP), bias_np


def _make_in_maps(x, w_np, mask_np, bias_np, tloc=TLOC, ncores=NCORES):
    xq = np.asarray(x, np.float32)[0].astype(_FP8_NP).astype(np.float32)
    xpad = np.pad(xq, ((0, 0), (AR, TB + 2)))       # [C, AR+T+TB+2]
    in_maps = []
    for i in range(ncores):
        lo = xpad[:, tloc * i : tloc * i + tloc + AR + 1]
        hi = xpad[:, tloc * i + 1 : tloc * i + tloc + AR + 2]
        # xed col u = x_glob[core_start + u]; row 64 ones; rows 65-127 zeros;
        # cols [tloc, tloc+TB) zero dead-zone for the s2 second k-tile
        xed_np = np.zeros((128, tloc + TB), np.float32)
        xed_np[0:C, 0:tloc] = xpad[:, tloc * i + AR : tloc * i + AR + tloc]
        xed_np[C, :] = 1.0
        xed_np[:, tloc:] = 0.0
        in_maps.append(
            {
                "xin": np.ascontiguousarray(
                    np.concatenate([lo, hi], axis=0).astype(_FP8_NP)
                ),
                "xed": xed_np.astype(_FP8_NP),
                "wts": w_np,
                "maskd": mask_np,
                "biasd": bias_np,
            }
        )
    return in_maps


def _run(x, W, b, Sigma, trace=False):
    if "nc" not in _CACHE:
        _CACHE["nc"] = _build_program()
    nc = _CACHE["nc"]
    w_np, mask_np, bias_np = _prep_host(
        np.asarray(W, np.float32), np.asarray(b, np.float32),
        np.asarray(Sigma, np.float32),
    )
    in_maps = _make_in_maps(np.asarray(x, np.float32), w_np, mask_np, bias_np)
    res = run_bass_kernel_spmd(
        nc, in_maps, core_ids=list(range(NCORES)), trace=trace
    )
    outs = [res.results[i]["out"] for i in range(NCORES)]
    full = np.concatenate(outs, axis=1)[None]   # [1, K, T]
    return full.astype(np.float32), res


def kernel(x, W, b, Sigma):
    out, _ = _run(x, W, b, Sigma, trace=bool(int(os.environ.get("BASS_TRACE", "0"))))
    return out
